# revision 3
# baseline (speedup 1.0000x reference)
"""Trainium2 Bass kernel for the conv-projected self-attention block.

Reference computation (B=8, C=256, N=64, K=256), per (b, n) slice X=[C,256]:
    q = wq X + bq; k = wk X + bk; v = wv X + bv
    s = k^T q / 16;  p = softmax(s, axis=l);  o = v p
    out = X + wp o + bp

This kernel restructures the math so only FOUR GEMM stages remain, all in
fp8 (e4m3) DoubleRow mode (contraction 256 in one PE pass, 2 cols/cycle):

  1. G-trick:  s = X^T (wk^T wq) X + beta_l + alpha_k + const.
     G = wk^T wq is host-precomputed; the per-query alpha_k and the constant
     cancel in softmax; beta_l = (wk^T bq)^T X is injected into the score
     PSUM with a rank-1 (K=1) fp8 matmul.  The K projection disappears.
  2. U-trick:  wp (v p^) = ((wp wv) X) p^ since row-sums of p^ scale per
     query and commute past wp.  U = (wp wv) X is computed transposed
     (X chunks stationary) so attention needs no on-chip transposes.  The
     V projection and the final projection collapse into one stage; bv
     folds into bp' = bp + wp bv, which is folded into the residual on host
     (xb = fp16(x + bp')).
  3. softmax: ep = exp(s/16 - 3) in fp8 (shift keeps exp in e4m3 range);
     sum via an all-(1/64) fp8 DR matmul (so 1/sum is scaled by 64, putting
     normalized probs in e4m3's normal range); epn = ep * (64/sum);
     final output scales by 1/64 in the eviction.

Sharding: data-parallel over B - one batch per NeuronCore (8 cores).
Residual + output ride fp16 (error ~1e-4, halves DMA).  Measured accuracy
vs the fp32 reference: rel_err ~1.1e-2 (dominated by e4m3 quantization).

Engine split per pair of n-slices: PE does scores/sum/UP DR matmuls +
projections; ACT does t-evictions + exp; DVE does recip, Ut-evictions and
the final affine+residual; GpSimd does the prob-normalize multiply
(SBUF-only op, keeping it off the PSUM-bound engines).
"""

import numpy as np
import ml_dtypes

import concourse.bass as bass
import concourse.bacc as bacc
import concourse.mybir as mybir
import concourse.tile as tile
from concourse.bass_utils import run_bass_kernel_spmd

F32 = mybir.dt.float32
F16 = mybir.dt.float16
FP8 = mybir.dt.float8e4
AF = mybir.ActivationFunctionType
ALU = mybir.AluOpType
DR = mybir.MatmulPerfMode.DoubleRow
E4 = ml_dtypes.float8_e4m3fn

B, C, N, K = 8, 256, 64, 256
GROUPS = [2, 6, 8, 8, 8, 8, 8, 8, 8]
assert sum(GROUPS) == N
SCALE = 1.0 / 16.0     # 1/sqrt(C) applied at exp
SHIFT = -3.0           # exp(s - 3): keeps ep in e4m3 range (max score ~7.7)
RSC = 64.0             # prob upscale; ones = 1/64 so recip = 64/sum

# which engine runs the prob-normalize multiply: "gpsimd" or "vector"
EPN_ENGINE = "gpsimd"

_CACHE = {}


def _build():
    nc = bacc.Bacc("TRN2", target_bir_lowering=False, debug=False,
                   num_devices=8)

    x8_d = nc.dram_tensor("x8", [C, N, K], FP8, kind="ExternalInput")
    xb_d = nc.dram_tensor("xb", [C, N, K], F16, kind="ExternalInput")
    gt_d = nc.dram_tensor("gt", [128, 2, C], FP8, kind="ExternalInput")
    wut_d = nc.dram_tensor("wut", [128, 2, C], FP8, kind="ExternalInput")
    bt_d = nc.dram_tensor("bt", [N, 2, 128], FP8, kind="ExternalInput")
    y_d = nc.dram_tensor("y", [C, N, K], F16, kind="ExternalOutput")

    with tile.TileContext(nc) as tc:
        with tc.tile_pool(name="const", bufs=1) as const, \
             tc.tile_pool(name="xg", bufs=2) as xgp, \
             tc.tile_pool(name="xh", bufs=2) as xhp, \
             tc.tile_pool(name="tg", bufs=2) as tgp, \
             tc.tile_pool(name="ut", bufs=2) as utp, \
             tc.tile_pool(name="sm", bufs=3) as smp, \
             tc.tile_pool(name="ot", bufs=3) as otp, \
             tc.tile_pool(name="ps_proj", bufs=1, space="PSUM") as ps_proj, \
             tc.tile_pool(name="ps_sc", bufs=2, space="PSUM") as ps_sc, \
             tc.tile_pool(name="ps_sum", bufs=1, space="PSUM") as ps_sum, \
             tc.tile_pool(name="ps_fin", bufs=1, space="PSUM") as ps_fin:

            # ---- first (small) X group loads before everything else ----
            G0 = GROUPS[0]
            xg_first = xgp.tile([128, 2, G0, K], FP8, name="xg", tag="xg")
            for ci in range(2):
                nc.sync.dma_start(out=xg_first[:, ci, :, :],
                                  in_=x8_d[bass.ts(ci, 128), 0:G0, :])

            # ---- constants ----
            gt = const.tile([128, 2, C], FP8, name="gt")
            wut = const.tile([128, 2, C], FP8, name="wut")
            bt = const.tile([1, N, 2, 128], FP8, name="bt")
            nc.sync.dma_start(out=gt[:, :, :], in_=gt_d[:, :, :])
            nc.sync.dma_start(out=wut[:, :, :], in_=wut_d[:, :, :])
            nc.sync.dma_start(out=bt[0:1, :, :, :], in_=bt_d[:, :, :])
            ones8 = const.tile([128, 2, 128], FP8, name="ones8")
            nc.vector.memset(ones8, 1.0 / RSC)
            onesr = const.tile([1, K], FP8, name="onesr")
            nc.vector.memset(onesr, 1.0)
            nbias = const.tile([128, 1], F32, name="nbias")
            nc.vector.memset(nbias, SHIFT)

            xh_first = xhp.tile([128, G0, 2, K], F16, name="xh", tag="xh")
            for ci in range(2):
                nc.sync.dma_start(out=xh_first[:, :, ci, :],
                                  in_=xb_d[bass.ts(ci, 128), 0:G0, :])

            n0 = 0
            for g, G in enumerate(GROUPS):
                if g == 0:
                    xg, xh = xg_first, xh_first
                else:
                    xg = xgp.tile([128, 2, G, K], FP8, name="xg", tag="xg")
                    xh = xhp.tile([128, G, 2, K], F16, name="xh", tag="xh")
                    for ci in range(2):
                        nc.sync.dma_start(
                            out=xg[:, ci, :, :],
                            in_=x8_d[bass.ts(ci, 128), n0:n0 + G, :])
                        nc.sync.dma_start(
                            out=xh[:, :, ci, :],
                            in_=xb_d[bass.ts(ci, 128), n0:n0 + G, :])

                # ---- t = G @ x projection: [128, 2(co), G, 256] fp8 ----
                # DR matmuls N=512; evict pairs of chunks 1024-wide on ACT
                tg = tgp.tile([128, 2, G, K], FP8, name="tg", tag="tg")
                nch = G // 2   # 512-wide chunks per co
                for co in range(2):
                    ch = 0
                    while ch < nch:
                        step = 2 if ch + 2 <= nch else 1
                        pst = ps_proj.tile([128, step, 512], F32, name="pst",
                                           tag="proj")
                        for j in range(step):
                            sl = slice(2 * (ch + j), 2 * (ch + j) + 2)
                            nc.tensor.matmul(
                                pst[:, j, :], gt[:, :, bass.ts(co, 128)],
                                xg[:, :, sl, :].rearrange("p a b c -> p a (b c)"),
                                start=True, stop=True, perf_mode=DR)
                        nc.scalar.copy(
                            tg[:, co, 2 * ch:2 * ch + 2 * step, :]
                            .rearrange("p a b -> p (a b)"),
                            pst.rearrange("p a b -> p (a b)"))
                        ch += step

                # ---- Ut = (wp wv x)^T projection: [128, 2G(pos), 256] ----
                # stationary = x8 chunks, DR, N=256; evict 4 pos 1024-wide DVE
                ut = utp.tile([128, 2 * G, C], FP8, name="ut", tag="ut")
                pt = 0
                while pt < 2 * G:
                    step = 4 if pt + 4 <= 2 * G else 2
                    psu = ps_proj.tile([128, step, 256], F32, name="psu",
                                       tag="proj")
                    for j in range(step):
                        p = pt + j
                        nc.tensor.matmul(
                            psu[:, j, :],
                            xg[:, :, p // 2, bass.ts(p % 2, 128)],
                            wut[:, :, :], start=True, stop=True, perf_mode=DR)
                    nc.vector.tensor_copy(
                        ut[:, pt:pt + step, :].rearrange("p a b -> p (a b)"),
                        psu.rearrange("p a b -> p (a b)"))
                    pt += step

                # ---- attention: pairs of slices ----
                for sp0 in range(0, G, 2):
                    # scores for both slices: pss [128, 2(lt), 2(sp), 256]
                    pss = ps_sc.tile([128, 2, 2, K], F32, name="pss", tag="sc")
                    for sp in range(2):
                        sl = sp0 + sp
                        for lt in range(2):
                            nc.tensor.matmul(
                                pss[:, lt, sp, :],
                                xg[:, :, sl, bass.ts(lt, 128)],
                                tg[:, :, sl, :],
                                start=True, stop=False, perf_mode=DR,
                                skip_group_check=True)
                            # beta_l rank-1 injection (K=1 fp8 matmul)
                            nc.tensor.matmul(
                                pss[:, lt, sp, :],
                                bt[0:1, n0 + sl, lt, :], onesr[0:1, :],
                                start=False, stop=True,
                                skip_group_check=True)

                    # ep = exp(s/16 - 3): one wide ACT op, fp8 out
                    ep = smp.tile([128, 2, 2, K], FP8, name="ep", tag="ep")
                    nc.scalar.activation(
                        out=ep.rearrange("p a b c -> p (a b c)"),
                        in_=pss.rearrange("p a b c -> p (a b c)"),
                        func=AF.Exp, scale=SCALE, bias=nbias[:, 0:1])

                    # sum_l ep / 64 for both slices; recip = 64/sum
                    psw = ps_sum.tile([128, 2, K], F32, name="psw", tag="sum")
                    nc.tensor.matmul(
                        psw.rearrange("p a b -> p (a b)"), ones8,
                        ep.rearrange("p a b c -> p a (b c)"),
                        start=True, stop=True, perf_mode=DR)
                    recip = smp.tile([128, 2, K], F32, name="recip",
                                     tag="recip")
                    nc.vector.reciprocal_approx_fast(out=recip, in_=psw)

                    # epn = ep * recip (normalized probs scaled by 64), fp8
                    epn = smp.tile([128, 2, 2, K], FP8, name="epn", tag="epn")
                    eng = nc.gpsimd if EPN_ENGINE == "gpsimd" else nc.vector
                    eng.tensor_tensor(
                        out=epn, in0=ep,
                        in1=recip.unsqueeze(1).broadcast_to((128, 2, 2, K)),
                        op=ALU.mult)

                    # out = U @ epn / 64 + xb ; psf [128, 2(ct), 256] / slice
                    outf = otp.tile([128, 2, 2, K], F16, name="outf",
                                    tag="outf")
                    for sp in range(2):
                        sl = sp0 + sp
                        psf = ps_fin.tile([128, 2, K], F32, name="psf",
                                          tag="fin")
                        for ct in range(2):
                            nc.tensor.matmul(
                                psf[:, ct, :],
                                ut[:, 2 * sl:2 * sl + 2, bass.ts(ct, 128)],
                                epn[:, :, sp, :],
                                start=True, stop=True, perf_mode=DR)
                        nc.vector.affine_then_add(
                            out=outf[:, sp, :, :].rearrange("p a b -> p (a b)"),
                            in0=psf.rearrange("p a b -> p (a b)"),
                            in1=xh[:, sl, :, :].rearrange("p a b -> p (a b)"),
                            scale=1.0 / RSC, bias=0.0)
                    for ct in range(2):
                        nc.sync.dma_start(
                            out=y_d[bass.ts(ct, 128),
                                    n0 + sp0:n0 + sp0 + 2, :],
                            in_=outf[:, :, ct, :])
                n0 += G

    nc.compile()
    return nc


def _get_nc():
    if "nc" not in _CACHE:
        _CACHE["nc"] = _build()
    return _CACHE["nc"]


def _host_prep(inputs):
    x = np.ascontiguousarray(np.asarray(inputs["x"]), dtype=np.float32)
    wq = np.asarray(inputs["wq"]).astype(np.float64)
    wk = np.asarray(inputs["wk"]).astype(np.float64)
    wv = np.asarray(inputs["wv"]).astype(np.float64)
    wp = np.asarray(inputs["wp"]).astype(np.float64)
    bq = np.asarray(inputs["bq"]).astype(np.float64)
    bv = np.asarray(inputs["bv"]).astype(np.float64)
    bp = np.asarray(inputs["bp"]).astype(np.float64)

    Gm = (wk.T @ wq).astype(np.float32)          # s = x^T G x (+beta)
    WU = (wp @ wv).astype(np.float32)            # out_pre = (WU x) p^
    bpe = (bp + wp @ bv).astype(np.float32)      # v/final bias, into residual
    bvec = (wk.T @ bq).astype(np.float32)        # beta_l = bvec . x_l

    def dr_stationary(M):   # [c, co] layouts -> [128, 2, 256] DR tiles
        return np.ascontiguousarray(
            M.reshape(2, 128, C).transpose(1, 0, 2))

    gt8 = dr_stationary(np.ascontiguousarray(Gm.T).astype(E4))
    wut8 = dr_stationary(np.ascontiguousarray(WU.T).astype(E4))

    x8 = x.astype(E4)                            # [B, C, N, K]
    xb = (x + bpe[None, :, None, None]).astype(np.float16)
    # beta[b, n, l] then [N, 2, 128] per core
    beta = np.einsum('c,bcnk->bnk', bvec, x).astype(E4)
    bt8 = np.ascontiguousarray(beta.reshape(B, N, 2, 128))
    return x8, xb, gt8, wut8, bt8


def run(inputs, trace=False):
    x8, xb, gt8, wut8, bt8 = _host_prep(inputs)
    nc = _get_nc()
    common = {"gt": gt8, "wut": wut8}
    in_maps = [dict(common, x8=x8[b], xb=xb[b], bt=bt8[b]) for b in range(B)]
    res = run_bass_kernel_spmd(nc, in_maps, core_ids=list(range(8)),
                               trace=trace)
    out = np.stack([res.results[b]["y"].astype(np.float32)
                    for b in range(B)], axis=0)
    return out, res


def kernel(**inputs):
    out, _ = run(inputs, trace=False)
    return out


# revision 9
# speedup vs baseline: 1.4064x; 1.4064x over previous
"""Trainium2 Bass kernel for the conv-projected self-attention block.

Reference computation (B=8, C=256, N=64, K=256), per (b, n) slice X=[C,256]:
    q = wq X + bq; k = wk X + bk; v = wv X + bv
    s = k^T q / 16;  p = softmax(s, axis=l);  o = v p
    out = X + wp o + bp

This kernel restructures the math so only FOUR GEMM stages remain, all in
fp8 (e4m3) DoubleRow mode (contraction 256 in one PE pass, 2 cols/cycle):

  1. G-trick:  s = X^T (wk^T wq) X + beta_l + alpha_k + const.
     G = wk^T wq is host-precomputed; the per-query alpha_k and the constant
     cancel in softmax; beta_l = (wk^T bq)^T X is injected into the score
     PSUM with a rank-1 (K=1) fp8 matmul.  The K projection disappears.
  2. U-trick:  wp (v p^) = ((wp wv) X) p^ since row-sums of p^ scale per
     query and commute past wp.  U = (wp wv) X is computed transposed
     (X chunks stationary) so attention needs no on-chip transposes.  The
     V projection and the final projection collapse into one stage; bv
     folds into bp' = bp + wp bv, which is folded into the residual on host
     (xb = fp16(x + bp')).
  3. softmax: ep = exp(s/16 - 3) in fp8 (shift keeps exp in e4m3 range);
     sum via an all-(1/64) fp8 DR matmul (so 1/sum is scaled by 64, putting
     normalized probs in e4m3's normal range); epn = ep * (64/sum);
     final output scales by 1/64 in the eviction.

Sharding: data-parallel over B - one batch per NeuronCore (8 cores).
Residual + output ride fp16 (error ~1e-4, halves DMA).  Measured accuracy
vs the fp32 reference: rel_err ~1.1e-2 (dominated by e4m3 quantization).

Engine split per pair of n-slices: PE does scores/sum/UP DR matmuls +
projections; ACT does t-evictions + exp; DVE does recip, Ut-evictions and
the final affine+residual; GpSimd does the prob-normalize multiply
(SBUF-only op, keeping it off the PSUM-bound engines).
"""

import numpy as np
import ml_dtypes

import concourse.bass as bass
import concourse.bacc as bacc
import concourse.mybir as mybir
import concourse.tile as tile
from concourse.bass_utils import run_bass_kernel_spmd

F32 = mybir.dt.float32
F16 = mybir.dt.float16
FP8 = mybir.dt.float8e4
AF = mybir.ActivationFunctionType
ALU = mybir.AluOpType
DR = mybir.MatmulPerfMode.DoubleRow
E4 = ml_dtypes.float8_e4m3fn

B, C, N, K = 8, 256, 64, 256
GROUPS = [2, 6, 8, 8, 8, 8, 8, 8, 8]
assert sum(GROUPS) == N
SCALE = 1.0 / 16.0     # 1/sqrt(C) applied at exp
SHIFT = -3.0           # exp(s - 3): keeps ep in e4m3 range (max score ~7.7)
RSC = 64.0             # prob upscale; ones = 1/64 so recip = 64/sum

# which engine runs the prob-normalize multiply: "gpsimd" or "vector"
EPN_ENGINE = "gpsimd"

_CACHE = {}


def _build():
    nc = bacc.Bacc("TRN2", target_bir_lowering=False, debug=False,
                   num_devices=8)

    x8_d = nc.dram_tensor("x8", [C, N, K], FP8, kind="ExternalInput")
    xb_d = nc.dram_tensor("xb", [C, N, K], F16, kind="ExternalInput")
    gt_d = nc.dram_tensor("gt", [128, 2, C], FP8, kind="ExternalInput")
    wut_d = nc.dram_tensor("wut", [128, 2, C], FP8, kind="ExternalInput")
    bt_d = nc.dram_tensor("bt", [32, N, 2, 2, 128], FP8, kind="ExternalInput")
    y_d = nc.dram_tensor("y", [C, N, K], F16, kind="ExternalOutput")

    with tile.TileContext(nc) as tc:
        with tc.tile_pool(name="const", bufs=1) as const, \
             tc.tile_pool(name="xg", bufs=2) as xgp, \
             tc.tile_pool(name="xh", bufs=2) as xhp, \
             tc.tile_pool(name="tg", bufs=2) as tgp, \
             tc.tile_pool(name="ut", bufs=2) as utp, \
             tc.tile_pool(name="sm", bufs=3) as smp, \
             tc.tile_pool(name="ot", bufs=3) as otp, \
             tc.tile_pool(name="ps_proj", bufs=2, space="PSUM") as ps_proj, \
             tc.tile_pool(name="ps_sc", bufs=2, space="PSUM") as ps_sc, \
             tc.tile_pool(name="ps_sum", bufs=1, space="PSUM") as ps_sum, \
             tc.tile_pool(name="ps_fin", bufs=1, space="PSUM") as ps_fin:

            # ---- first (small) X group loads before everything else ----
            G0 = GROUPS[0]
            xg_first = xgp.tile([128, 2, G0, K], FP8, name="xg", tag="xg")
            for ci in range(2):
                nc.sync.dma_start(out=xg_first[:, ci, :, :],
                                  in_=x8_d[bass.ts(ci, 128), 0:G0, :])

            # ---- constants ----
            gt = const.tile([128, 2, C], FP8, name="gt")
            wut = const.tile([128, 2, C], FP8, name="wut")
            bt = const.tile([32, N, 2, 2, 128], FP8, name="bt")
            nc.sync.dma_start(out=gt[:, :, :], in_=gt_d[:, :, :])
            nc.sync.dma_start(out=wut[:, :, :], in_=wut_d[:, :, :])
            nc.sync.dma_start(out=bt[:, :, :, :, :], in_=bt_d[:, :, :, :, :])
            ones8 = const.tile([128, 2, 128], FP8, name="ones8")
            nc.vector.memset(ones8, 1.0 / RSC)
            onesr = const.tile([32, 2, K], FP8, name="onesr")
            nc.vector.memset(onesr, 1.0)
            nbias = const.tile([128, 1], F32, name="nbias")
            nc.vector.memset(nbias, SHIFT)

            xh_first = xhp.tile([128, G0, 2, K], F16, name="xh", tag="xh")
            for ci in range(2):
                nc.sync.dma_start(out=xh_first[:, :, ci, :],
                                  in_=xb_d[bass.ts(ci, 128), 0:G0, :])

            n0 = 0
            for g, G in enumerate(GROUPS):
                if g == 0:
                    xg, xh = xg_first, xh_first
                else:
                    xg = xgp.tile([128, 2, G, K], FP8, name="xg", tag="xg")
                    xh = xhp.tile([128, G, 2, K], F16, name="xh", tag="xh")
                    for ci in range(2):
                        nc.sync.dma_start(
                            out=xg[:, ci, :, :],
                            in_=x8_d[bass.ts(ci, 128), n0:n0 + G, :])
                        nc.sync.dma_start(
                            out=xh[:, :, ci, :],
                            in_=xb_d[bass.ts(ci, 128), n0:n0 + G, :])

                # ---- t = G @ x projection: [128, 2(co), G, 256] fp8 ----
                # DR matmuls N=512; evict pairs of chunks 1024-wide on ACT
                tg = tgp.tile([128, 2, G, K], FP8, name="tg", tag="tg")
                nch = G // 2   # 512-wide chunks per co
                for co in range(2):
                    ch = 0
                    while ch < nch:
                        step = 2 if ch + 2 <= nch else 1
                        pst = ps_proj.tile([128, step, 512], F32, name="pst",
                                           tag="proj")
                        for j in range(step):
                            sl = slice(2 * (ch + j), 2 * (ch + j) + 2)
                            nc.tensor.matmul(
                                pst[:, j, :], gt[:, :, bass.ts(co, 128)],
                                xg[:, :, sl, :].rearrange("p a b c -> p a (b c)"),
                                start=True, stop=True, perf_mode=DR)
                        nc.scalar.copy(
                            tg[:, co, 2 * ch:2 * ch + 2 * step, :]
                            .rearrange("p a b -> p (a b)"),
                            pst.rearrange("p a b -> p (a b)"))
                        ch += step

                # ---- Ut = (wp wv x)^T projection: [128, 2G(pos), 256] ----
                # stationary = x8 chunks, DR, N=256; evict 4 pos 1024-wide DVE
                ut = utp.tile([128, 2 * G, C], FP8, name="ut", tag="ut")
                pt = 0
                uev = 0
                while pt < 2 * G:
                    step = 4 if pt + 4 <= 2 * G else 2
                    psu = ps_proj.tile([128, step, 256], F32, name="psu",
                                       tag="proj")
                    for j in range(step):
                        p = pt + j
                        nc.tensor.matmul(
                            psu[:, j, :],
                            xg[:, :, p // 2, bass.ts(p % 2, 128)],
                            wut[:, :, :], start=True, stop=True, perf_mode=DR)
                    dst = ut[:, pt:pt + step, :].rearrange("p a b -> p (a b)")
                    srcp = psu.rearrange("p a b -> p (a b)")
                    if uev % 3 == 0:
                        nc.scalar.copy(dst, srcp)
                    else:
                        nc.vector.tensor_copy(dst, srcp)
                    uev += 1
                    pt += step

                # ---- attention: pairs of slices ----
                for sp0 in range(0, G, 2):
                    # scores per slice: pss [128, 2(lt), 256].  beta rides
                    # as a rank-1 DoubleRow matmul (zero 2nd k-tile) so the
                    # PE never switches perf modes.  NOTE: a DR start=True
                    # clears has_written for the whole PSUM bank, so each
                    # region's start->stop pair must stay adjacent -- no
                    # other start=True into the same bank in between.
                    ep = smp.tile([128, 2, 2, K], FP8, name="ep", tag="ep")
                    for sp in range(2):
                        sl = sp0 + sp
                        pss = ps_sc.tile([128, 2, K], F32, name="pss",
                                         tag="sc")
                        for lt in range(2):
                            nc.tensor.matmul(
                                pss[:, lt, :],
                                xg[:, :, sl, bass.ts(lt, 128)],
                                tg[:, :, sl, :],
                                start=True, stop=False, perf_mode=DR,
                                skip_group_check=True)
                            nc.tensor.matmul(
                                pss[:, lt, :],
                                bt[:, n0 + sl, lt, :, :],
                                onesr[:, :, :],
                                start=False, stop=True, perf_mode=DR,
                                skip_group_check=True)
                        nc.scalar.activation(
                            out=ep[:, :, sp, :], in_=pss,
                            func=AF.Exp, scale=SCALE, bias=nbias[:, 0:1])

                    # sum_l ep / 64 for both slices; recip = 64/sum
                    psw = ps_sum.tile([128, 2, K], F32, name="psw", tag="sum")
                    nc.tensor.matmul(
                        psw.rearrange("p a b -> p (a b)"), ones8,
                        ep.rearrange("p a b c -> p a (b c)"),
                        start=True, stop=True, perf_mode=DR)
                    recip = smp.tile([128, 2, K], F32, name="recip",
                                     tag="recip")
                    nc.vector.reciprocal_approx_fast(out=recip, in_=psw)

                    # epn = ep * recip (normalized probs scaled by 64), fp8
                    epn = smp.tile([128, 2, 2, K], FP8, name="epn", tag="epn")
                    eng = nc.gpsimd if EPN_ENGINE == "gpsimd" else nc.vector
                    eng.tensor_tensor(
                        out=epn, in0=ep,
                        in1=recip.unsqueeze(1).broadcast_to((128, 2, 2, K)),
                        op=ALU.mult)

                    # out = U @ epn / 64 + xb ; psf [128, 2(ct), 256] / slice
                    outf = otp.tile([128, 2, 2, K], F16, name="outf",
                                    tag="outf")
                    for sp in range(2):
                        sl = sp0 + sp
                        psf = ps_fin.tile([128, 2, K], F32, name="psf",
                                          tag="fin")
                        for ct in range(2):
                            nc.tensor.matmul(
                                psf[:, ct, :],
                                ut[:, 2 * sl:2 * sl + 2, bass.ts(ct, 128)],
                                epn[:, :, sp, :],
                                start=True, stop=True, perf_mode=DR)
                        nc.vector.affine_then_add(
                            out=outf[:, sp, :, :].rearrange("p a b -> p (a b)"),
                            in0=psf.rearrange("p a b -> p (a b)"),
                            in1=xh[:, sl, :, :].rearrange("p a b -> p (a b)"),
                            scale=1.0 / RSC, bias=0.0)
                    for ct in range(2):
                        nc.sync.dma_start(
                            out=y_d[bass.ts(ct, 128),
                                    n0 + sp0:n0 + sp0 + 2, :],
                            in_=outf[:, :, ct, :])
                n0 += G

    nc.compile()
    return nc


def _get_nc():
    if "nc" not in _CACHE:
        _CACHE["nc"] = _build()
    return _CACHE["nc"]


def _host_prep(inputs):
    x = np.ascontiguousarray(np.asarray(inputs["x"]), dtype=np.float32)
    wq = np.asarray(inputs["wq"]).astype(np.float64)
    wk = np.asarray(inputs["wk"]).astype(np.float64)
    wv = np.asarray(inputs["wv"]).astype(np.float64)
    wp = np.asarray(inputs["wp"]).astype(np.float64)
    bq = np.asarray(inputs["bq"]).astype(np.float64)
    bv = np.asarray(inputs["bv"]).astype(np.float64)
    bp = np.asarray(inputs["bp"]).astype(np.float64)

    Gm = (wk.T @ wq).astype(np.float32)          # s = x^T G x (+beta)
    WU = (wp @ wv).astype(np.float32)            # out_pre = (WU x) p^
    bpe = (bp + wp @ bv).astype(np.float32)      # v/final bias, into residual
    bvec = (wk.T @ bq).astype(np.float32)        # beta_l = bvec . x_l

    def dr_stationary(M):   # [c, co] layouts -> [128, 2, 256] DR tiles
        return np.ascontiguousarray(
            M.reshape(2, 128, C).transpose(1, 0, 2))

    gt8 = dr_stationary(np.ascontiguousarray(Gm.T).astype(E4))
    wut8 = dr_stationary(np.ascontiguousarray(WU.T).astype(E4))

    x8 = x.astype(E4)                            # [B, C, N, K]
    xb = (x + bpe[None, :, None, None]).astype(np.float16)
    # beta[b, n, l] then [N, 2, 128] per core
    beta = np.einsum('c,bcnk->bnk', bvec, x).astype(E4)
    bt8 = np.zeros((B, 32, N, 2, 2, 128), dtype=E4)
    bt8[:, 0, :, :, 0, :] = beta.reshape(B, N, 2, 128)
    return x8, xb, gt8, wut8, bt8


def run(inputs, trace=False):
    x8, xb, gt8, wut8, bt8 = _host_prep(inputs)
    nc = _get_nc()
    common = {"gt": gt8, "wut": wut8}
    in_maps = [dict(common, x8=x8[b], xb=xb[b], bt=bt8[b]) for b in range(B)]
    res = run_bass_kernel_spmd(nc, in_maps, core_ids=list(range(8)),
                               trace=trace)
    out = np.stack([res.results[b]["y"].astype(np.float32)
                    for b in range(B)], axis=0)
    return out, res


def kernel(**inputs):
    out, _ = run(inputs, trace=False)
    return out


# revision 13
# speedup vs baseline: 1.5104x; 1.0740x over previous
"""Trainium2 Bass kernel for the conv-projected self-attention block.

Reference computation (B=8, C=256, N=64, K=256), per (b, n) slice X=[C,256]:
    q = wq X + bq; k = wk X + bk; v = wv X + bv
    s = k^T q / 16;  p = softmax(s, axis=l);  o = v p
    out = X + wp o + bp

This kernel restructures the math so only FOUR GEMM stages remain, all in
fp8 (e4m3) DoubleRow mode (contraction 256 in one PE pass, 2 cols/cycle):

  1. G-trick:  s = X^T (wk^T wq) X + beta_l + alpha_k + const.
     G = wk^T wq is host-precomputed; the per-query alpha_k and the constant
     cancel in softmax; beta_l = (wk^T bq)^T X is injected into the score
     PSUM with a rank-1 (K=1) fp8 matmul.  The K projection disappears.
  2. U-trick:  wp (v p^) = ((wp wv) X) p^ since row-sums of p^ scale per
     query and commute past wp.  U = (wp wv) X is computed transposed
     (X chunks stationary) so attention needs no on-chip transposes.  The
     V projection and the final projection collapse into one stage; bv
     folds into bp' = bp + wp bv, which is folded into the residual on host
     (xb = fp16(x + bp')).
  3. softmax: ep = exp(s/16 - 3) in fp8 (shift keeps exp in e4m3 range);
     sum via an all-(1/64) fp8 DR matmul (so 1/sum is scaled by 64, putting
     normalized probs in e4m3's normal range); epn = ep * (64/sum);
     final output scales by 1/64 in the eviction.

Sharding: data-parallel over B - one batch per NeuronCore (8 cores).
Residual + output ride fp16 (error ~1e-4, halves DMA).  Measured accuracy
vs the fp32 reference: rel_err ~1.1e-2 (dominated by e4m3 quantization).

Engine split per pair of n-slices: PE does scores/sum/UP DR matmuls +
projections; ACT does t-evictions + exp; DVE does recip, Ut-evictions and
the final affine+residual; GpSimd does the prob-normalize multiply
(SBUF-only op, keeping it off the PSUM-bound engines).
"""

import numpy as np
import ml_dtypes

import concourse.bass as bass
import concourse.bacc as bacc
import concourse.mybir as mybir
import concourse.tile as tile
from concourse.bass_utils import run_bass_kernel_spmd

F32 = mybir.dt.float32
F16 = mybir.dt.float16
FP8 = mybir.dt.float8e4
AF = mybir.ActivationFunctionType
ALU = mybir.AluOpType
DR = mybir.MatmulPerfMode.DoubleRow
E4 = ml_dtypes.float8_e4m3fn

B, C, N, K = 8, 256, 64, 256
GROUPS = [2, 6, 8, 8, 8, 8, 8, 8, 8]
assert sum(GROUPS) == N
SCALE = 1.0 / 16.0     # 1/sqrt(C) applied at exp
SHIFT = -3.0           # exp(s - 3): keeps ep in e4m3 range (max score ~7.7)
RSC = 64.0             # prob upscale; ones = 1/64 so recip = 64/sum

# which engine runs the prob-normalize multiply: "gpsimd" or "vector"
EPN_ENGINE = "gpsimd"

_CACHE = {}


def _build():
    nc = bacc.Bacc("TRN2", target_bir_lowering=False, debug=False,
                   num_devices=8)

    x8_d = nc.dram_tensor("x8", [C, N, K], FP8, kind="ExternalInput")
    xb_d = nc.dram_tensor("xb", [C, N, K], F16, kind="ExternalInput")
    gt_d = nc.dram_tensor("gt", [128, 2, C], FP8, kind="ExternalInput")
    wut_d = nc.dram_tensor("wut", [128, 2, C], FP8, kind="ExternalInput")
    bt_d = nc.dram_tensor("bt", [32, N // 2, 2, 2, 128], FP8, kind="ExternalInput")
    mk_d = nc.dram_tensor("mk", [32, 2, 2, K], FP8, kind="ExternalInput")
    y_d = nc.dram_tensor("y", [C, N, K], F16, kind="ExternalOutput")

    with tile.TileContext(nc) as tc:
        with tc.tile_pool(name="const", bufs=1) as const, \
             tc.tile_pool(name="xg", bufs=2) as xgp, \
             tc.tile_pool(name="xh", bufs=2) as xhp, \
             tc.tile_pool(name="tg", bufs=2) as tgp, \
             tc.tile_pool(name="ut", bufs=2) as utp, \
             tc.tile_pool(name="sm", bufs=3) as smp, \
             tc.tile_pool(name="ot", bufs=3) as otp, \
             tc.tile_pool(name="ps_proj", bufs=2, space="PSUM") as ps_proj, \
             tc.tile_pool(name="ps_sc", bufs=1, space="PSUM") as ps_sc, \
             tc.tile_pool(name="ps_sum", bufs=1, space="PSUM") as ps_sum, \
             tc.tile_pool(name="ps_fin", bufs=1, space="PSUM") as ps_fin:

            # ---- first (small) X group loads before everything else ----
            G0 = GROUPS[0]
            xg_first = xgp.tile([128, 2, G0, K], FP8, name="xg", tag="xg")
            for ci in range(2):
                nc.sync.dma_start(out=xg_first[:, ci, :, :],
                                  in_=x8_d[bass.ts(ci, 128), 0:G0, :])

            # ---- constants ----
            gt = const.tile([128, 2, C], FP8, name="gt")
            wut = const.tile([128, 2, C], FP8, name="wut")
            bt = const.tile([32, N // 2, 2, 2, 128], FP8, name="bt")
            nc.sync.dma_start(out=gt[:, :, :], in_=gt_d[:, :, :])
            nc.sync.dma_start(out=wut[:, :, :], in_=wut_d[:, :, :])
            nc.sync.dma_start(out=bt[:, :, :, :, :], in_=bt_d[:, :, :, :, :])
            ones8 = const.tile([128, 2, 128], FP8, name="ones8")
            nc.vector.memset(ones8, 1.0 / RSC)
            # mask[0,:,0,:]=1 selects slice0's beta row, mask[1,:,1,:]=1
            # selects slice1's; all other rows zero (host-prepared)
            maskr = const.tile([32, 2, 2, K], FP8, name="maskr")
            nc.sync.dma_start(out=maskr[:, :, :, :], in_=mk_d[:, :, :, :])
            nbias = const.tile([128, 1], F32, name="nbias")
            nc.vector.memset(nbias, SHIFT)

            xh_first = xhp.tile([128, G0, 2, K], F16, name="xh", tag="xh")
            for ci in range(2):
                nc.sync.dma_start(out=xh_first[:, :, ci, :],
                                  in_=xb_d[bass.ts(ci, 128), 0:G0, :])

            n0 = 0
            for g, G in enumerate(GROUPS):
                if g == 0:
                    xg, xh = xg_first, xh_first
                else:
                    xg = xgp.tile([128, 2, G, K], FP8, name="xg", tag="xg")
                    xh = xhp.tile([128, G, 2, K], F16, name="xh", tag="xh")
                    for ci in range(2):
                        nc.sync.dma_start(
                            out=xg[:, ci, :, :],
                            in_=x8_d[bass.ts(ci, 128), n0:n0 + G, :])
                        nc.sync.dma_start(
                            out=xh[:, :, ci, :],
                            in_=xb_d[bass.ts(ci, 128), n0:n0 + G, :])

                # ---- t = G @ x projection: [128, 2(co), G, 256] fp8 ----
                # DR matmuls N=512; evict pairs of chunks 1024-wide on ACT
                tg = tgp.tile([128, 2, G, K], FP8, name="tg", tag="tg")
                nch = G // 2   # 512-wide chunks per co
                for co in range(2):
                    ch = 0
                    while ch < nch:
                        step = 2 if ch + 2 <= nch else 1
                        pst = ps_proj.tile([128, step, 512], F32, name="pst",
                                           tag="proj")
                        for j in range(step):
                            sl = slice(2 * (ch + j), 2 * (ch + j) + 2)
                            nc.tensor.matmul(
                                pst[:, j, :], gt[:, :, bass.ts(co, 128)],
                                xg[:, :, sl, :].rearrange("p a b c -> p a (b c)"),
                                start=True, stop=True, perf_mode=DR)
                        nc.scalar.copy(
                            tg[:, co, 2 * ch:2 * ch + 2 * step, :]
                            .rearrange("p a b -> p (a b)"),
                            pst.rearrange("p a b -> p (a b)"))
                        ch += step

                # ---- Ut = (wp wv x)^T projection: [128, 2G(pos), 256] ----
                # stationary = x8 chunks, DR, N=256; evict 4 pos 1024-wide DVE
                ut = utp.tile([128, 2 * G, C], FP8, name="ut", tag="ut")
                pt = 0
                uev = 0
                while pt < 2 * G:
                    step = 4 if pt + 4 <= 2 * G else 2
                    psu = ps_proj.tile([128, step, 256], F32, name="psu",
                                       tag="proj")
                    for j in range(step):
                        p = pt + j
                        nc.tensor.matmul(
                            psu[:, j, :],
                            xg[:, :, p // 2, bass.ts(p % 2, 128)],
                            wut[:, :, :], start=True, stop=True, perf_mode=DR)
                    dst = ut[:, pt:pt + step, :].rearrange("p a b -> p (a b)")
                    srcp = psu.rearrange("p a b -> p (a b)")
                    if uev % 2 == 0:
                        nc.scalar.copy(dst, srcp)
                    else:
                        nc.vector.tensor_copy(dst, srcp)
                    uev += 1
                    pt += step

                # ---- attention: pairs of slices ----
                for sp0 in range(0, G, 2):
                    # scores for the pair: pss [128, 2(lt), 2(sp), 256].
                    # Bank = lt; a DR start=True clears has_written for the
                    # whole bank, so per bank: slice0 DR uses start=True,
                    # slice1 DR start=False (bank already pending-zero =>
                    # overwrite), then ONE rank-2 beta matmul (N=512)
                    # accumulates both slices' beta via the 0/1 mask.
                    pss = ps_sc.tile([128, 2, 2, K], F32, name="pss",
                                     tag="sc")
                    for lt in range(2):
                        for sp in range(2):
                            nc.tensor.matmul(
                                pss[:, lt, sp, :],
                                xg[:, :, sp0 + sp, bass.ts(lt, 128)],
                                tg[:, :, sp0 + sp, :],
                                start=(sp == 0), stop=False, perf_mode=DR,
                                skip_group_check=True)
                        nc.tensor.matmul(
                            pss[:, lt, :, :].rearrange("p a b -> p (a b)"),
                            bt[:, (n0 + sp0) // 2, lt, :, :],
                            maskr.rearrange("p a b c -> p a (b c)"),
                            start=False, stop=True, perf_mode=DR,
                            skip_group_check=True)
                    ep = smp.tile([128, 2, 2, K], FP8, name="ep", tag="ep")
                    nc.scalar.activation(
                        out=ep.rearrange("p a b c -> p (a b c)"),
                        in_=pss.rearrange("p a b c -> p (a b c)"),
                        func=AF.Exp, scale=SCALE, bias=nbias[:, 0:1])

                    # sum_l ep / 64 for both slices; recip = 64/sum
                    psw = ps_sum.tile([128, 2, K], F32, name="psw", tag="sum")
                    nc.tensor.matmul(
                        psw.rearrange("p a b -> p (a b)"), ones8,
                        ep.rearrange("p a b c -> p a (b c)"),
                        start=True, stop=True, perf_mode=DR)
                    recip = smp.tile([128, 2, K], F32, name="recip",
                                     tag="recip")
                    nc.vector.reciprocal_approx_fast(out=recip, in_=psw)

                    # epn = ep * recip (normalized probs scaled by 64), fp8
                    epn = smp.tile([128, 2, 2, K], FP8, name="epn", tag="epn")
                    eng = nc.gpsimd if EPN_ENGINE == "gpsimd" else nc.vector
                    eng.tensor_tensor(
                        out=epn, in0=ep,
                        in1=recip.unsqueeze(1).broadcast_to((128, 2, 2, K)),
                        op=ALU.mult)

                    # out = U @ epn / 64 + xb ; psf [128, 2(ct), 256] / slice
                    outf = otp.tile([128, 2, 2, K], F16, name="outf",
                                    tag="outf")
                    for sp in range(2):
                        sl = sp0 + sp
                        psf = ps_fin.tile([128, 2, K], F32, name="psf",
                                          tag="fin")
                        for ct in range(2):
                            nc.tensor.matmul(
                                psf[:, ct, :],
                                ut[:, 2 * sl:2 * sl + 2, bass.ts(ct, 128)],
                                epn[:, :, sp, :],
                                start=True, stop=True, perf_mode=DR)
                        nc.vector.affine_then_add(
                            out=outf[:, sp, :, :].rearrange("p a b -> p (a b)"),
                            in0=psf.rearrange("p a b -> p (a b)"),
                            in1=xh[:, sl, :, :].rearrange("p a b -> p (a b)"),
                            scale=1.0 / RSC, bias=0.0)
                    for ct in range(2):
                        nc.sync.dma_start(
                            out=y_d[bass.ts(ct, 128),
                                    n0 + sp0:n0 + sp0 + 2, :],
                            in_=outf[:, :, ct, :])
                n0 += G

    nc.compile()
    return nc


def _get_nc():
    if "nc" not in _CACHE:
        _CACHE["nc"] = _build()
    return _CACHE["nc"]


def _host_prep(inputs):
    x = np.ascontiguousarray(np.asarray(inputs["x"]), dtype=np.float32)
    wq = np.asarray(inputs["wq"]).astype(np.float64)
    wk = np.asarray(inputs["wk"]).astype(np.float64)
    wv = np.asarray(inputs["wv"]).astype(np.float64)
    wp = np.asarray(inputs["wp"]).astype(np.float64)
    bq = np.asarray(inputs["bq"]).astype(np.float64)
    bv = np.asarray(inputs["bv"]).astype(np.float64)
    bp = np.asarray(inputs["bp"]).astype(np.float64)

    Gm = (wk.T @ wq).astype(np.float32)          # s = x^T G x (+beta)
    WU = (wp @ wv).astype(np.float32)            # out_pre = (WU x) p^
    bpe = (bp + wp @ bv).astype(np.float32)      # v/final bias, into residual
    bvec = (wk.T @ bq).astype(np.float32)        # beta_l = bvec . x_l

    def dr_stationary(M):   # [c, co] layouts -> [128, 2, 256] DR tiles
        return np.ascontiguousarray(
            M.reshape(2, 128, C).transpose(1, 0, 2))

    gt8 = dr_stationary(np.ascontiguousarray(Gm.T).astype(E4))
    wut8 = dr_stationary(np.ascontiguousarray(WU.T).astype(E4))

    x8 = x.astype(E4)                            # [B, C, N, K]
    xb = (x + bpe[None, :, None, None]).astype(np.float16)
    # beta[b, n, l] then [N, 2, 128] per core
    beta = np.einsum('c,bcnk->bnk', bvec, x).astype(E4)
    bt8 = np.zeros((B, 32, N // 2, 2, 2, 128), dtype=E4)
    br = beta.reshape(B, N // 2, 2, 2, 128)   # [b, pair, sp, lt, l]
    bt8[:, 0, :, :, 0, :] = br[:, :, 0, :, :]
    bt8[:, 1, :, :, 0, :] = br[:, :, 1, :, :]
    mk8 = np.zeros((32, 2, 2, K), dtype=E4)
    mk8[0, :, 0, :] = 1.0
    mk8[1, :, 1, :] = 1.0
    return x8, xb, gt8, wut8, bt8, mk8


def run(inputs, trace=False):
    x8, xb, gt8, wut8, bt8, mk8 = _host_prep(inputs)
    nc = _get_nc()
    common = {"gt": gt8, "wut": wut8, "mk": mk8}
    in_maps = [dict(common, x8=x8[b], xb=xb[b], bt=bt8[b]) for b in range(B)]
    res = run_bass_kernel_spmd(nc, in_maps, core_ids=list(range(8)),
                               trace=trace)
    out = np.stack([res.results[b]["y"].astype(np.float32)
                    for b in range(B)], axis=0)
    return out, res


def kernel(**inputs):
    out, _ = run(inputs, trace=False)
    return out
